# revision 15
# baseline (speedup 1.0000x reference)
"""Trainium2 Bass kernel for nn_AppearanceBlock (self-attention block).

Reference computation (per batch sample b, N = H*W = 4096):
    q = Wq @ pose + bq      [32, N]
    k = Wk @ src  + bk      [32, N]
    v = Wv @ src  + bv      [256, N]
    att = softmax(q^T k, axis=-1)        [N, N]
    out = gamma * (v @ att^T) + src

Distribution: pure data-parallel — 8 cores = 4 batch samples x 2 query
halves (m in [half*2048, half*2048+2048)). Each core gets the full
source[b] (for k, v) and its pose/source m-slice; no collectives.

Layout: the whole attention pipeline runs in "transposed" layout so no
on-chip transposes are needed:
    energyT[n, m] = sum_o k[o,n] q[o,m]      (n on partitions)
    expT = exp(energyT)                      (bf16, ScalarE, PSUM->SBUF)
    out[c, m] = sum_n vT[n,c] expT[n,m]      (PSUM accumulate over n)
    rowsum[m] = sum_n 1 * expT[n,m]          (ones-vector matmul)
    out = (gamma/rowsum)*AV + (src + gamma*bv)    (bv folds in because
          sum_n bv[c]*expT[n,m] = bv[c]*rowsum[m])
vT[n, c] comes directly from the v-projection with the source tile as
the stationary operand. Softmax max-subtraction is skipped: |energy| is
bounded (~25) so exp stays in range, identically to the shifted form.

PE-array packing (the q/k contraction dim is only 32):
 - k is computed "stacked": column-group g of the PE writes k for the
   n-range [1024g, 1024g+1024) to partitions [32g, 32g+32). q is
   computed 4x replicated the same way. The energy matmuls then run as
   4 concurrent row-group matmuls (tile_position=(32g, 0)).
 - rowsum runs as 4 concurrent M=1 column-group matmuls whose partials
   land on partitions {0,32,64,96} and are summed by 3 vector adds.
Energy matmuls for chunk mc+1 are emitted interleaved into the AV
stream of chunk mc so the PE never waits on the exp activation.

All matmuls are bf16 (inputs pre-cast on host); accumulation is fp32 in
PSUM and the epilogue/residual path is fp32.
"""

import numpy as np
import ml_dtypes

from contextlib import ExitStack

import concourse.bass as bass
import concourse.tile as tile
from concourse import mybir, bacc
from concourse.bass_utils import run_bass_kernel_spmd

B, C, H, W = 4, 256, 64, 64
N = H * W            # 4096 keys per sample
CQ = C // 8          # 32 q/k channels
NCORES = 8
MLOC = N * B // NCORES   # 2048 queries per core
P = 128
MCHUNK = 512
NMC = MLOC // MCHUNK     # 4 m-chunks
NT = N // P              # 32 n-tiles
CT = C // P              # 2 c-tiles
NG = 4                   # PE row/col groups
GN = N // NG             # 1024 n per group

F32 = mybir.dt.float32
BF16 = mybir.dt.bfloat16
AF = mybir.ActivationFunctionType

TRACE = False
LAST_RESULT = None
_CACHED_NC = None


def build_graph():
    nc = bacc.Bacc()

    s_d = nc.declare_dram_parameter("s", [C, N], BF16, isOutput=False)
    p_d = nc.declare_dram_parameter("p", [C, MLOC], BF16, isOutput=False)
    src_d = nc.declare_dram_parameter("src", [C, MLOC], F32, isOutput=False)
    wqt_d = nc.declare_dram_parameter("wqt", [C, CQ], BF16, isOutput=False)
    wkt_d = nc.declare_dram_parameter("wkt", [C, CQ], BF16, isOutput=False)
    wvt_d = nc.declare_dram_parameter("wvt", [C, C], BF16, isOutput=False)
    bqr_d = nc.declare_dram_parameter("bqr", [P, 1], F32, isOutput=False)
    bkr_d = nc.declare_dram_parameter("bkr", [P, 1], F32, isOutput=False)
    bv_d = nc.declare_dram_parameter("bv", [P, CT], F32, isOutput=False)
    gam_d = nc.declare_dram_parameter("gam", [1, 1], F32, isOutput=False)
    out_d = nc.declare_dram_parameter("out", [C, MLOC], F32, isOutput=True)

    s_ap = s_d[:].rearrange("(co p) n -> p co n", p=P)       # [128, 2, 4096]
    p_ap = p_d[:].rearrange("(co p) m -> p co m", p=P)       # [128, 2, 2048]
    src_ap = src_d[:].rearrange("(co p) m -> p co m", p=P)
    wqt_ap = wqt_d[:].rearrange("(co p) o -> p co o", p=P)   # [128, 2, 32]
    wkt_ap = wkt_d[:].rearrange("(co p) o -> p co o", p=P)
    wvt_ap = wvt_d[:].rearrange("(co p) c -> p co c", p=P)   # [128, 2, 256]
    out_ap = out_d[:].rearrange("(co p) m -> p co m", p=P)

    with tile.TileContext(nc) as tc, ExitStack() as ctx:
        const = ctx.enter_context(tc.tile_pool(name="const", bufs=1))
        big = ctx.enter_context(tc.tile_pool(name="big", bufs=1))

        # ---- persistent input loads ----
        # Small weights first, then p (q path), then s (k/vt path), then
        # src (only needed at the epilogue). s and p are loaded into
        # separate 512-wide tiles so matmuls depend only on their slice.
        wqt_sb = const.tile([P, CT, CQ], BF16)
        nc.sync.dma_start(wqt_sb[:], wqt_ap)
        wkt_sb = const.tile([P, CT, CQ], BF16)
        nc.sync.dma_start(wkt_sb[:], wkt_ap)
        wvt_sb = const.tile([P, CT, C], BF16)
        nc.sync.dma_start(wvt_sb[:], wvt_ap)
        bqr_sb = const.tile([P, 1], F32)
        nc.sync.dma_start(bqr_sb[:], bqr_d[:])
        bkr_sb = const.tile([P, 1], F32)
        nc.sync.dma_start(bkr_sb[:], bkr_d[:])
        bv_sb = const.tile([P, CT], F32)
        nc.sync.dma_start(bv_sb[:], bv_d[:])
        gam_sb = const.tile([1, 1], F32)
        nc.sync.dma_start(gam_sb[:], gam_d[:])

        p_tiles = []
        for i in range(MLOC // MCHUNK):
            pt = big.tile([P, CT, MCHUNK], BF16, tag=f"p_{i}", name=f"p_{i}")
            nc.sync.dma_start(pt[:], p_ap[:, :, i * MCHUNK:(i + 1) * MCHUNK])
            p_tiles.append(pt)
        s_tiles = []
        for i in range(N // MCHUNK):
            st = big.tile([P, CT, MCHUNK], BF16, tag=f"s_{i}", name=f"s_{i}")
            nc.sync.dma_start(st[:], s_ap[:, :, i * MCHUNK:(i + 1) * MCHUNK])
            s_tiles.append(st)
        src_sb = big.tile([P, CT, MLOC], F32)
        for i in range(4):
            sl = slice(i * (MLOC // 4), (i + 1) * (MLOC // 4))
            nc.sync.dma_start(src_sb[:, :, sl], src_ap[:, :, sl])

        ones_bf = const.tile([P, 1], BF16)
        nc.any.memset(ones_bf[:], 1.0)

        # gamma broadcast to all partitions; gbv = gamma * bv
        gamb_sb = const.tile([P, 1], F32)
        nc.gpsimd.partition_broadcast(gamb_sb[:], gam_sb[:])
        gbv_sb = const.tile([P, CT], F32)
        nc.vector.tensor_scalar_mul(gbv_sb[:], bv_sb[:], gamb_sb[:])

        # q replicated to 4 partition groups; k stacked by n-group
        q_st = big.tile([P, MLOC], BF16)
        k_st = big.tile([P, GN], BF16)
        vt_sb = big.tile([P, NT, C], BF16)

        # ---- projections (their PSUM pool closes before the main loop) ----
        with tc.tile_pool(name="pjps", bufs=2, space="PSUM") as pjps:
            # q: same [32, 512] result written to 4 col groups
            for mc in range(NMC):
                sl = slice(mc * MCHUNK, (mc + 1) * MCHUNK)
                qp = pjps.tile([P, MCHUNK], F32, tag="pj")
                for g in range(NG):
                    for co in range(CT):
                        nc.tensor.matmul(qp[32 * g:32 * (g + 1), :],
                                         wqt_sb[:, co, :], p_tiles[mc][:, co, :],
                                         start=(co == 0), stop=(co == CT - 1),
                                         tile_position=(0, 32 * g))
                nc.scalar.activation(q_st[:, sl], qp[:], AF.Identity,
                                     bias=bqr_sb[:])
            # k: col group g holds n-range [1024g, 1024g+1024)
            for u in range(GN // MCHUNK):
                kp = pjps.tile([P, GN], F32, tag="pjk")
                for g in range(NG):
                    for co in range(CT):
                        nc.tensor.matmul(kp[32 * g:32 * (g + 1),
                                            u * MCHUNK:(u + 1) * MCHUNK],
                                         wkt_sb[:, co, :],
                                         s_tiles[2 * g + u][:, co, :],
                                         start=(co == 0), stop=(co == CT - 1),
                                         tile_position=(0, 32 * g))
                nc.scalar.activation(
                    k_st[:, u * MCHUNK:(u + 1) * MCHUNK],
                    kp[:, u * MCHUNK:(u + 1) * MCHUNK],
                    AF.Identity, bias=bkr_sb[:])

        # ---- attention: software-pipelined over m-chunks ----
        eps_pool = ctx.enter_context(tc.tile_pool(name="eps", bufs=1, space="PSUM"))
        exp_pool = ctx.enter_context(tc.tile_pool(name="expt", bufs=2))
        outp = ctx.enter_context(tc.tile_pool(name="outp", bufs=3))
        small = ctx.enter_context(tc.tile_pool(name="small", bufs=4))

        exp_tiles = {}

        def emit_energy_slot(mc, s8):
            """4 concurrent row-group matmuls + one exp for n-tiles
            {8g + s8 : g in 0..3} of chunk mc."""
            sl = slice(mc * MCHUNK, (mc + 1) * MCHUNK)
            if s8 == 0:
                exp_tiles[mc] = exp_pool.tile([P, NT, MCHUNK], BF16, tag="expT", name=f"expT_{mc}")
            expT = exp_tiles[mc]
            eps = eps_pool.tile([P, NG, MCHUNK], F32, tag="eps", name=f"eps_{mc}_{s8}")
            for g in range(NG):
                nc.tensor.matmul(eps[:, g, :],
                                 k_st[32 * g:32 * (g + 1),
                                      s8 * P:(s8 + 1) * P],
                                 q_st[32 * g:32 * (g + 1), sl],
                                 start=True, stop=True,
                                 tile_position=(32 * g, 0))
            nc.scalar.activation(expT[:, s8::NT // NG, :], eps[:], AF.Exp)

        # vT projection interleaved with chunk-0 energy slots: the PE does
        # vt matmuls while ScalarE drains the chunk-0 exp calls.
        with tc.tile_pool(name="vtps", bufs=2, space="PSUM") as vtps:
            for t in range(NT):
                vp = vtps.tile([P, C], F32, tag="vp", name=f"vp_{t}")
                toff = (t % 4) * P
                for co in range(CT):
                    nc.tensor.matmul(vp[:],
                                     s_tiles[t // 4][:, co, toff:toff + P],
                                     wvt_sb[:, co, :],
                                     start=(co == 0), stop=(co == CT - 1))
                nc.vector.tensor_scalar_mul(vt_sb[:, t, :], vp[:], gamb_sb[:])
                if t % 4 == 1:
                    emit_energy_slot(0, t // 4)

        av_pool = ctx.enter_context(tc.tile_pool(name="av", bufs=3, space="PSUM"))
        rs_pool = ctx.enter_context(tc.tile_pool(name="rs", bufs=1, space="PSUM"))

        for mc in range(NMC):
            sl = slice(mc * MCHUNK, (mc + 1) * MCHUNK)
            expT = exp_tiles[mc]
            av0 = av_pool.tile([P, MCHUNK], F32, tag="av")
            av1 = av_pool.tile([P, MCHUNK], F32, tag="av")
            rs = rs_pool.tile([P, MCHUNK], F32, tag="rs")
            def emit_rs_slot(u):
                for j in range(NG):
                    nc.tensor.matmul(rs[32 * j:32 * j + 1, :], ones_bf[:],
                                     expT[:, 4 * u + j, :],
                                     start=(u == 0), stop=(u == NT // 4 - 1),
                                     tile_position=(0, 32 * j))

            def emit_combine():
                rsum = small.tile([1, MCHUNK], F32, tag="rsum",
                                  name=f"rsum_{mc}")
                nc.vector.tensor_copy(rsum[:], rs[0:1, :])
                nc.vector.tensor_add(rsum[:], rsum[:], rs[32:33, :])
                nc.vector.tensor_add(rsum[:], rsum[:], rs[64:65, :])
                nc.vector.tensor_add(rsum[:], rsum[:], rs[96:97, :])
                recip = small.tile([1, MCHUNK], F32, tag="rc",
                                   name=f"recip_{mc}")
                nc.vector.reciprocal_approx_fast(recip[:], rsum[:])
                recipb = small.tile([P, MCHUNK], F32, tag="rb",
                                    name=f"recipb_{mc}")
                nc.gpsimd.partition_broadcast(recipb[:], recip[:])
                return recipb

            recipb = None
            for t in range(NT):
                st, sp = (t == 0), (t == NT - 1)
                nc.tensor.matmul(av0[:], vt_sb[:, t, 0:P], expT[:, t, :],
                                 start=st, stop=sp)
                nc.tensor.matmul(av1[:], vt_sb[:, t, P:C], expT[:, t, :],
                                 start=st, stop=sp)
                if t % 4 == 1 and mc + 1 < NMC:
                    emit_energy_slot(mc + 1, t // 4)
                if mc == 0:
                    if t % 4 == 3:
                        emit_rs_slot(t // 4)
                else:
                    # expT(mc) is fully available: finish rowsum early and
                    # hide the combine/recip/broadcast under the AV stream
                    if 3 <= t <= 10:
                        emit_rs_slot(t - 3)
                    elif t == 12:
                        recipb = emit_combine()
            if recipb is None:
                recipb = emit_combine()
            HM = MCHUNK // 2
            for h in range(2):
                hs = slice(h * HM, (h + 1) * HM)
                gs = slice(mc * MCHUNK + h * HM, mc * MCHUNK + (h + 1) * HM)
                for co, av in ((0, av0), (1, av1)):
                    o = outp.tile([P, HM], F32, tag="o")
                    nc.vector.tensor_mul(o[:], av[:, hs], recipb[:, hs])
                    nc.vector.tensor_add(o[:], o[:], src_sb[:, co, gs])
                    nc.vector.tensor_scalar_add(o[:], o[:],
                                                gbv_sb[:, co:co + 1])
                    nc.sync.dma_start(out_ap[:, co, gs], o[:])

    nc.compile()
    return nc


def _get_nc():
    global _CACHED_NC
    if _CACHED_NC is None:
        _CACHED_NC = build_graph()
    return _CACHED_NC


def kernel(**inputs):
    global LAST_RESULT
    source = np.ascontiguousarray(np.asarray(inputs["source"], dtype=np.float32))
    pose = np.ascontiguousarray(np.asarray(inputs["pose"], dtype=np.float32))
    Wq = np.asarray(inputs["Wq"], dtype=np.float32)
    bq = np.asarray(inputs["bq"], dtype=np.float32)
    Wk = np.asarray(inputs["Wk"], dtype=np.float32)
    bk = np.asarray(inputs["bk"], dtype=np.float32)
    Wv = np.asarray(inputs["Wv"], dtype=np.float32)
    bv = np.asarray(inputs["bv"], dtype=np.float32)
    gamma = np.asarray(inputs["gamma"], dtype=np.float32)

    bf = ml_dtypes.bfloat16
    s_all = source.reshape(B, C, N)
    p_all = pose.reshape(B, C, N)
    s_bf = s_all.astype(bf)
    p_bf = p_all.astype(bf)
    wqt = np.ascontiguousarray(Wq.T.astype(bf))
    wkt = np.ascontiguousarray(Wk.T.astype(bf))
    wvt = np.ascontiguousarray(Wv.T.astype(bf))
    bqr = np.ascontiguousarray(np.tile(bq, P // CQ).reshape(P, 1))
    bkr = np.ascontiguousarray(np.tile(bk, P // CQ).reshape(P, 1))
    bvr = np.ascontiguousarray(bv.reshape(CT, P).T)
    gam = gamma.reshape(1, 1)

    in_maps = []
    for core in range(NCORES):
        b, half = core // 2, core % 2
        msl = slice(half * MLOC, (half + 1) * MLOC)
        in_maps.append({
            "s": np.ascontiguousarray(s_bf[b]),
            "p": np.ascontiguousarray(p_bf[b][:, msl]),
            "src": np.ascontiguousarray(s_all[b][:, msl]),
            "wqt": wqt, "wkt": wkt, "wvt": wvt,
            "bqr": bqr, "bkr": bkr, "bv": bvr, "gam": gam,
        })

    nc = _get_nc()
    res = run_bass_kernel_spmd(nc, in_maps, core_ids=list(range(NCORES)),
                               trace=TRACE)
    LAST_RESULT = res

    out = np.empty((B, C, N), dtype=np.float32)
    for core in range(NCORES):
        b, half = core // 2, core % 2
        out[b][:, half * MLOC:(half + 1) * MLOC] = res.results[core]["out"]
    return out.reshape(B, C, H, W)


# revision 17
# speedup vs baseline: 1.1213x; 1.1213x over previous
"""Trainium2 Bass kernel for nn_AppearanceBlock (self-attention block).

Reference computation (per batch sample b, N = H*W = 4096):
    q = Wq @ pose + bq      [32, N]
    k = Wk @ src  + bk      [32, N]
    v = Wv @ src  + bv      [256, N]
    att = softmax(q^T k, axis=-1)        [N, N]
    out = gamma * (v @ att^T) + src

Distribution: pure data-parallel — 8 cores = 4 batch samples x 2 query
halves (m in [half*2048, half*2048+2048)). Each core gets the full
source[b] (for k, v) and its pose/source m-slice; no collectives.

Layout: the whole attention pipeline runs in "transposed" layout so no
on-chip transposes are needed:
    energyT[n, m] = sum_o k[o,n] q[o,m]      (n on partitions)
    expT = exp(energyT)                      (bf16, ScalarE, PSUM->SBUF)
    out[c, m] = sum_n vT[n,c] expT[n,m]      (PSUM accumulate over n)
    rowsum[m] = sum_n 1 * expT[n,m]          (ones-vector matmul)
    out = (gamma/rowsum)*AV + (src + gamma*bv)    (bv folds in because
          sum_n bv[c]*expT[n,m] = bv[c]*rowsum[m])
vT[n, c] comes directly from the v-projection with the source tile as
the stationary operand. Softmax max-subtraction is skipped: |energy| is
bounded (~25) so exp stays in range, identically to the shifted form.

PE-array packing (the q/k contraction dim is only 32):
 - k is computed "stacked": column-group g of the PE writes k for the
   n-range [1024g, 1024g+1024) to partitions [32g, 32g+32). q is
   computed 4x replicated the same way. The energy matmuls then run as
   4 concurrent row-group matmuls (tile_position=(32g, 0)).
 - rowsum runs as 4 concurrent M=1 column-group matmuls whose partials
   land on partitions {0,32,64,96} and are summed by 3 vector adds.
Energy matmuls for chunk mc+1 are emitted interleaved into the AV
stream of chunk mc so the PE never waits on the exp activation.

All matmuls are bf16 (inputs pre-cast on host); accumulation is fp32 in
PSUM and the epilogue/residual path is fp32.
"""

import numpy as np
import ml_dtypes

from contextlib import ExitStack

import concourse.bass as bass
import concourse.tile as tile
from concourse import mybir, bacc
from concourse.bass_utils import run_bass_kernel_spmd

B, C, H, W = 4, 256, 64, 64
N = H * W            # 4096 keys per sample
CQ = C // 8          # 32 q/k channels
NCORES = 8
MLOC = N * B // NCORES   # 2048 queries per core
P = 128
MCHUNK = 512
NMC = MLOC // MCHUNK     # 4 m-chunks
NT = N // P              # 32 n-tiles
CT = C // P              # 2 c-tiles
NG = 4                   # PE row/col groups
GN = N // NG             # 1024 n per group

F32 = mybir.dt.float32
BF16 = mybir.dt.bfloat16
AF = mybir.ActivationFunctionType

TRACE = False
LAST_RESULT = None
_CACHED_NC = None


def build_graph():
    nc = bacc.Bacc()

    s_d = nc.declare_dram_parameter("s", [C, N], BF16, isOutput=False)
    p_d = nc.declare_dram_parameter("p", [C, MLOC], BF16, isOutput=False)
    src_d = nc.declare_dram_parameter("src", [C, MLOC], F32, isOutput=False)
    wqt_d = nc.declare_dram_parameter("wqt", [C, CQ], BF16, isOutput=False)
    wkt_d = nc.declare_dram_parameter("wkt", [C, CQ], BF16, isOutput=False)
    wvt_d = nc.declare_dram_parameter("wvt", [C, C], BF16, isOutput=False)
    bqr_d = nc.declare_dram_parameter("bqr", [P, 1], F32, isOutput=False)
    bkr_d = nc.declare_dram_parameter("bkr", [P, 1], F32, isOutput=False)
    bv_d = nc.declare_dram_parameter("bv", [P, CT], F32, isOutput=False)
    gam_d = nc.declare_dram_parameter("gam", [1, 1], F32, isOutput=False)
    out_d = nc.declare_dram_parameter("out", [C, MLOC], F32, isOutput=True)

    s_ap = s_d[:].rearrange("(co p) n -> p co n", p=P)       # [128, 2, 4096]
    p_ap = p_d[:].rearrange("(co p) m -> p co m", p=P)       # [128, 2, 2048]
    src_ap = src_d[:].rearrange("(co p) m -> p co m", p=P)
    wqt_ap = wqt_d[:].rearrange("(co p) o -> p co o", p=P)   # [128, 2, 32]
    wkt_ap = wkt_d[:].rearrange("(co p) o -> p co o", p=P)
    wvt_ap = wvt_d[:].rearrange("(co p) c -> p co c", p=P)   # [128, 2, 256]
    out_ap = out_d[:].rearrange("(co p) m -> p co m", p=P)

    with tile.TileContext(nc) as tc, ExitStack() as ctx:
        const = ctx.enter_context(tc.tile_pool(name="const", bufs=1))
        big = ctx.enter_context(tc.tile_pool(name="big", bufs=1))

        # ---- persistent input loads ----
        # Small weights first, then p (q path), then s (k/vt path), then
        # src (only needed at the epilogue). s and p are loaded into
        # separate 512-wide tiles so matmuls depend only on their slice.
        wqt_sb = const.tile([P, CT, CQ], BF16)
        nc.sync.dma_start(wqt_sb[:], wqt_ap)
        wkt_sb = const.tile([P, CT, CQ], BF16)
        nc.sync.dma_start(wkt_sb[:], wkt_ap)
        bqr_sb = const.tile([P, 1], F32)
        nc.sync.dma_start(bqr_sb[:], bqr_d[:])
        bkr_sb = const.tile([P, 1], F32)
        nc.sync.dma_start(bkr_sb[:], bkr_d[:])
        p_tiles = []
        for i in range(MLOC // MCHUNK):
            pt = big.tile([P, CT, MCHUNK], BF16, tag=f"p_{i}", name=f"p_{i}")
            nc.sync.dma_start(pt[:], p_ap[:, :, i * MCHUNK:(i + 1) * MCHUNK])
            p_tiles.append(pt)
        s_tiles = []
        for i in range(4):
            st = big.tile([P, CT, MCHUNK], BF16, tag=f"s_{i}", name=f"s_{i}")
            nc.sync.dma_start(st[:], s_ap[:, :, i * MCHUNK:(i + 1) * MCHUNK])
            s_tiles.append(st)
        wvt_sb = const.tile([P, CT, C], BF16)
        nc.sync.dma_start(wvt_sb[:], wvt_ap)
        for i in range(4, 8):
            st = big.tile([P, CT, MCHUNK], BF16, tag=f"s_{i}", name=f"s_{i}")
            nc.sync.dma_start(st[:], s_ap[:, :, i * MCHUNK:(i + 1) * MCHUNK])
            s_tiles.append(st)
        bv_sb = const.tile([P, CT], F32)
        nc.sync.dma_start(bv_sb[:], bv_d[:])
        gam_sb = const.tile([1, 1], F32)
        nc.sync.dma_start(gam_sb[:], gam_d[:])
        src_sb = big.tile([P, CT, MLOC], F32)
        for i in range(4):
            sl = slice(i * (MLOC // 4), (i + 1) * (MLOC // 4))
            nc.sync.dma_start(src_sb[:, :, sl], src_ap[:, :, sl])

        ones_bf = const.tile([P, 1], BF16)
        nc.any.memset(ones_bf[:], 1.0)
        warm = const.tile([1, 1], F32)
        nc.scalar.activation(warm[:], ones_bf[:1, :], AF.Exp)

        # gamma broadcast to all partitions; gbv = gamma * bv
        gamb_sb = const.tile([P, 1], F32)
        nc.gpsimd.partition_broadcast(gamb_sb[:], gam_sb[:])
        gbv_sb = const.tile([P, CT], F32)
        nc.vector.tensor_scalar_mul(gbv_sb[:], bv_sb[:], gamb_sb[:])

        # q replicated to 4 partition groups; k stacked by n-group
        q_st = big.tile([P, MLOC], BF16)
        k_st = big.tile([P, GN], BF16)
        vt_sb = big.tile([P, NT, C], BF16)

        # ---- projections (their PSUM pool closes before the main loop) ----
        with tc.tile_pool(name="pjps", bufs=2, space="PSUM") as pjps:
            # q: same [32, 512] result written to 4 col groups
            for mc in range(NMC):
                sl = slice(mc * MCHUNK, (mc + 1) * MCHUNK)
                qp = pjps.tile([P, MCHUNK], F32, tag="pj")
                for g in range(NG):
                    for co in range(CT):
                        nc.tensor.matmul(qp[32 * g:32 * (g + 1), :],
                                         wqt_sb[:, co, :], p_tiles[mc][:, co, :],
                                         start=(co == 0), stop=(co == CT - 1),
                                         tile_position=(0, 32 * g))
                nc.scalar.activation(q_st[:, sl], qp[:], AF.Identity,
                                     bias=bqr_sb[:])
            # k: col group g holds n-range [1024g, 1024g+1024)
            for u in range(GN // MCHUNK):
                kp = pjps.tile([P, GN], F32, tag="pjk")
                for g in range(NG):
                    for co in range(CT):
                        nc.tensor.matmul(kp[32 * g:32 * (g + 1),
                                            u * MCHUNK:(u + 1) * MCHUNK],
                                         wkt_sb[:, co, :],
                                         s_tiles[2 * g + u][:, co, :],
                                         start=(co == 0), stop=(co == CT - 1),
                                         tile_position=(0, 32 * g))
                nc.scalar.activation(
                    k_st[:, u * MCHUNK:(u + 1) * MCHUNK],
                    kp[:, u * MCHUNK:(u + 1) * MCHUNK],
                    AF.Identity, bias=bkr_sb[:])

        # ---- attention: software-pipelined over m-chunks ----
        eps_pool = ctx.enter_context(tc.tile_pool(name="eps", bufs=1, space="PSUM"))
        exp_pool = ctx.enter_context(tc.tile_pool(name="expt", bufs=3))
        outp = ctx.enter_context(tc.tile_pool(name="outp", bufs=3))
        small = ctx.enter_context(tc.tile_pool(name="small", bufs=4))

        exp_tiles = {}

        def emit_energy_slot(mc, s8):
            """4 concurrent row-group matmuls + one exp for n-tiles
            {8g + s8 : g in 0..3} of chunk mc."""
            sl = slice(mc * MCHUNK, (mc + 1) * MCHUNK)
            if s8 == 0:
                exp_tiles[mc] = exp_pool.tile([P, NT, MCHUNK], BF16, tag="expT", name=f"expT_{mc}")
            expT = exp_tiles[mc]
            eps = eps_pool.tile([P, NG, MCHUNK], F32, tag="eps", name=f"eps_{mc}_{s8}")
            for g in range(NG):
                nc.tensor.matmul(eps[:, g, :],
                                 k_st[32 * g:32 * (g + 1),
                                      s8 * P:(s8 + 1) * P],
                                 q_st[32 * g:32 * (g + 1), sl],
                                 start=True, stop=True,
                                 tile_position=(32 * g, 0))
            nc.scalar.activation(expT[:, s8::NT // NG, :], eps[:], AF.Exp)

        # vT projection interleaved with chunk-0 energy slots: the PE does
        # vt matmuls while ScalarE drains the chunk-0 exp calls.
        with tc.tile_pool(name="vtps", bufs=2, space="PSUM") as vtps:
            for t in range(NT):
                vp = vtps.tile([P, C], F32, tag="vp", name=f"vp_{t}")
                toff = (t % 4) * P
                for co in range(CT):
                    nc.tensor.matmul(vp[:],
                                     s_tiles[t // 4][:, co, toff:toff + P],
                                     wvt_sb[:, co, :],
                                     start=(co == 0), stop=(co == CT - 1))
                nc.vector.tensor_scalar_mul(vt_sb[:, t, :], vp[:], gamb_sb[:])
                if t % 4 == 1:
                    emit_energy_slot(0, t // 4)

        av_pool = ctx.enter_context(tc.tile_pool(name="av", bufs=3, space="PSUM"))
        rs_pool = ctx.enter_context(tc.tile_pool(name="rs", bufs=1, space="PSUM"))

        for mc in range(NMC):
            sl = slice(mc * MCHUNK, (mc + 1) * MCHUNK)
            expT = exp_tiles[mc]
            av0 = av_pool.tile([P, MCHUNK], F32, tag="av")
            av1 = av_pool.tile([P, MCHUNK], F32, tag="av")
            rs = rs_pool.tile([P, MCHUNK], F32, tag="rs")
            for t in range(NT):
                st, sp = (t == 0), (t == NT - 1)
                nc.tensor.matmul(av0[:], vt_sb[:, t, 0:P], expT[:, t, :],
                                 start=st, stop=sp)
                nc.tensor.matmul(av1[:], vt_sb[:, t, P:C], expT[:, t, :],
                                 start=st, stop=sp)
                if t % 4 == 1 and mc + 1 < NMC:
                    emit_energy_slot(mc + 1, t // 4)
                if t % 4 == 3:
                    u = t // 4
                    for j in range(NG):
                        tt = 4 * u + j
                        nc.tensor.matmul(rs[32 * j:32 * j + 1, :], ones_bf[:],
                                         expT[:, tt, :],
                                         start=(u == 0), stop=(u == NT // 4 - 1),
                                         tile_position=(0, 32 * j))
            # epilogue: out = (gamma/rowsum)*AV + src + gamma*bv
            rsum = small.tile([1, MCHUNK], F32, tag="rsum")
            nc.vector.tensor_copy(rsum[:], rs[0:1, :])
            nc.vector.tensor_add(rsum[:], rsum[:], rs[32:33, :])
            nc.vector.tensor_add(rsum[:], rsum[:], rs[64:65, :])
            nc.vector.tensor_add(rsum[:], rsum[:], rs[96:97, :])
            recip = small.tile([1, MCHUNK], F32, tag="rc")
            nc.vector.reciprocal_approx_fast(recip[:], rsum[:])
            recipb = small.tile([P, MCHUNK], F32, tag="rb")
            nc.gpsimd.partition_broadcast(recipb[:], recip[:])
            HM = MCHUNK // 2
            for h in range(2):
                hs = slice(h * HM, (h + 1) * HM)
                gs = slice(mc * MCHUNK + h * HM, mc * MCHUNK + (h + 1) * HM)
                for co, av in ((0, av0), (1, av1)):
                    o = outp.tile([P, HM], F32, tag="o")
                    nc.vector.tensor_mul(o[:], av[:, hs], recipb[:, hs])
                    nc.vector.tensor_add(o[:], o[:], src_sb[:, co, gs])
                    nc.vector.tensor_scalar_add(o[:], o[:],
                                                gbv_sb[:, co:co + 1])
                    nc.sync.dma_start(out_ap[:, co, gs], o[:])

    nc.compile()
    return nc


def _get_nc():
    global _CACHED_NC
    if _CACHED_NC is None:
        _CACHED_NC = build_graph()
    return _CACHED_NC


def kernel(**inputs):
    global LAST_RESULT
    source = np.ascontiguousarray(np.asarray(inputs["source"], dtype=np.float32))
    pose = np.ascontiguousarray(np.asarray(inputs["pose"], dtype=np.float32))
    Wq = np.asarray(inputs["Wq"], dtype=np.float32)
    bq = np.asarray(inputs["bq"], dtype=np.float32)
    Wk = np.asarray(inputs["Wk"], dtype=np.float32)
    bk = np.asarray(inputs["bk"], dtype=np.float32)
    Wv = np.asarray(inputs["Wv"], dtype=np.float32)
    bv = np.asarray(inputs["bv"], dtype=np.float32)
    gamma = np.asarray(inputs["gamma"], dtype=np.float32)

    bf = ml_dtypes.bfloat16
    s_all = source.reshape(B, C, N)
    p_all = pose.reshape(B, C, N)
    s_bf = s_all.astype(bf)
    p_bf = p_all.astype(bf)
    wqt = np.ascontiguousarray(Wq.T.astype(bf))
    wkt = np.ascontiguousarray(Wk.T.astype(bf))
    wvt = np.ascontiguousarray(Wv.T.astype(bf))
    bqr = np.ascontiguousarray(np.tile(bq, P // CQ).reshape(P, 1))
    bkr = np.ascontiguousarray(np.tile(bk, P // CQ).reshape(P, 1))
    bvr = np.ascontiguousarray(bv.reshape(CT, P).T)
    gam = gamma.reshape(1, 1)

    in_maps = []
    for core in range(NCORES):
        b, half = core // 2, core % 2
        msl = slice(half * MLOC, (half + 1) * MLOC)
        in_maps.append({
            "s": np.ascontiguousarray(s_bf[b]),
            "p": np.ascontiguousarray(p_bf[b][:, msl]),
            "src": np.ascontiguousarray(s_all[b][:, msl]),
            "wqt": wqt, "wkt": wkt, "wvt": wvt,
            "bqr": bqr, "bkr": bkr, "bv": bvr, "gam": gam,
        })

    nc = _get_nc()
    res = run_bass_kernel_spmd(nc, in_maps, core_ids=list(range(NCORES)),
                               trace=TRACE)
    LAST_RESULT = res

    out = np.empty((B, C, N), dtype=np.float32)
    for core in range(NCORES):
        b, half = core // 2, core % 2
        out[b][:, half * MLOC:(half + 1) * MLOC] = res.results[core]["out"]
    return out.reshape(B, C, H, W)
